# revision 1
# baseline (speedup 1.0000x reference)
"""Trainium2 Bass kernel for nn_Causal_Temporal_Map_Attention_2.

Reference computation (B=16, N=2048, T=512, E=64):
    W0m  = relu(triu(W0, 1))
    attn = (x@W0m.T)@x.T + (x@W1.T)@e.T + (e@W2.T)@x.T + (e@W3.T)@e.T
    out  = attn @ x

Associativity lets us avoid the [B, N, N] attention map entirely:
    G[b] = x[b].T @ x[b]                  # [512, 512]
    H[b] = e.T @ x[b]                     # [64, 512]
    M0[b] = W0m.T @ G[b] + W1.T @ H[b]    # [512, 512]
    M1[b] = W2.T @ G[b] + W3.T @ H[b]     # [64, 512]
    out[b] = x[b] @ M0[b] + e @ M1[b]     # [2048, 512]

Sharding: data-parallel over batch, 2 batches per core on 8 cores.
Matmuls run in fp32r (tf32) on the PE: full-rate at N=512, ~2e-4 rel err.
"""

import numpy as np

import concourse.bass as bass
import concourse.mybir as mybir
import concourse.tile as tile
from concourse import bacc
from concourse.bass import ts
from concourse.bass_utils import run_bass_kernel_spmd
from concourse.masks import make_identity

N_CORES = 8
B = 16
B2 = B // N_CORES  # batches per core
N = 2048
T = 512
E = 64
NCHUNKS = N // 128  # 16
KT = T // 128  # 4

f32 = mybir.dt.float32
f32r = mybir.dt.float32r
AF = mybir.ActivationFunctionType


def build_module(iters: int = 1):
    nc = bacc.Bacc("TRN2", target_bir_lowering=False, debug=False, num_devices=N_CORES)

    X = nc.dram_tensor("x", [B2, N, T], f32, kind="ExternalInput").ap()
    Ein = nc.dram_tensor("e", [N, E], f32, kind="ExternalInput").ap()
    W0 = nc.dram_tensor("W0", [T, T], f32, kind="ExternalInput").ap()
    W1 = nc.dram_tensor("W1", [E, T], f32, kind="ExternalInput").ap()
    W2 = nc.dram_tensor("W2", [T, E], f32, kind="ExternalInput").ap()
    W3 = nc.dram_tensor("W3", [E, E], f32, kind="ExternalInput").ap()
    OUT = nc.dram_tensor("out", [B2, N, T], f32, kind="ExternalOutput").ap()

    with tile.TileContext(nc) as tc:
        with (
            tc.tile_pool(name="const", bufs=1) as cpool,
            tc.tile_pool(name="wstage", bufs=1) as wpool,
            tc.tile_pool(name="xstage", bufs=8) as xspool,
            tc.tile_pool(name="xr", bufs=2) as xrpool,
            tc.tile_pool(name="gh", bufs=1) as ghpool,
            tc.tile_pool(name="m01", bufs=1) as mpool,
            tc.tile_pool(name="xt", bufs=4) as xtpool,
            tc.tile_pool(name="outst", bufs=4) as opool,
            # "acc" tag is shared by stage B/C accumulators and stage D out
            # tiles: 5 banks rotating.  "pt" transposes get 3 banks.
            tc.tile_pool(name="psacc", bufs=5, space="PSUM") as psacc,
            tc.tile_pool(name="pst", bufs=3, space="PSUM") as pst,
        ):
            # ---- batch 0 x loads stream first: stage B is the first PE work
            def load_x(b):
                xr = [
                    xrpool.tile([128, T], f32r, tag=f"xr{i}", name=f"xr{i}")
                    for i in range(NCHUNKS)
                ]
                for i in range(NCHUNKS):
                    xs = xspool.tile([128, T], f32, tag="xst")
                    nc.sync.dma_start(xs[:], X[b, ts(i, 128), :])
                    nc.scalar.activation(xr[i][:], xs[:], AF.Copy)
                return xr

            xr_b = load_x(0)

            # e after x on the queue: H/eT don't need it until ~25us in
            e_st = wpool.tile([128, NCHUNKS, E], f32)
            nc.sync.dma_start(e_st[:], Ein.rearrange("(a p) k -> p a k", p=128))
            er = cpool.tile([128, NCHUNKS, E], f32r)
            nc.vector.tensor_copy(er[:], e_st[:])

            ident32 = cpool.tile([128, 128], f32)
            make_identity(nc, ident32[:])
            identr = cpool.tile([128, 128], f32r)
            nc.vector.tensor_copy(identr[:], ident32[:])

            # eT: [64, 2048] as 4 tiles of [64, 512]
            etr = [
                cpool.tile([E, 512], f32r, tag=f"etr{g}", name=f"etr{g}")
                for g in range(4)
            ]
            for g in range(4):
                pte = pst.tile([128, 512], f32r, tag="pt")
                for j in range(4):
                    i = g * 4 + j
                    nc.tensor.transpose(pte[:E, ts(j, 128)], er[:, i, :], identr[:])
                nc.vector.tensor_copy(etr[g][:], pte[:E, :])

            # ---- weights (only needed by stage C — after the first G pass)
            w0_st = wpool.tile([128, KT, T], f32)
            for kt in range(KT):
                nc.sync.dma_start(w0_st[:, kt, :], W0[ts(kt, 128), :])
            w0mr = cpool.tile([128, KT, T], f32r)
            for kt in range(KT):
                # keep W0[d, t] iff t >= d+1  (d = p + 128*kt)
                nc.gpsimd.affine_select(
                    out=w0_st[:, kt, :],
                    in_=w0_st[:, kt, :],
                    compare_op=mybir.AluOpType.is_ge,
                    fill=0.0,
                    base=-(128 * kt + 1),
                    pattern=[[1, T]],
                    channel_multiplier=-1,
                )
                nc.scalar.activation(w0mr[:, kt, :], w0_st[:, kt, :], AF.Relu)

            w1_st = wpool.tile([E, T], f32)
            nc.sync.dma_start(w1_st[:], W1[:])
            w1r = cpool.tile([E, T], f32r)
            nc.vector.tensor_copy(w1r[:], w1_st[:])

            w2_st = wpool.tile([128, KT, E], f32)
            for kt in range(KT):
                nc.sync.dma_start(w2_st[:, kt, :], W2[ts(kt, 128), :])
            w2r = cpool.tile([128, KT, E], f32r)
            nc.vector.tensor_copy(w2r[:], w2_st[:])

            w3_st = wpool.tile([E, E], f32)
            nc.sync.dma_start(w3_st[:], W3[:])
            w3r = cpool.tile([E, E], f32r)
            nc.vector.tensor_copy(w3r[:], w3_st[:])

            # ---------------- per batch ----------------
            for it in range(iters):
                for b in range(B2):
                    xr = xr_b

                    # Stage B: G = x^T x (4 chunks), H = e^T x.
                    # k-outer: each arriving x tile feeds 5 matmuls.
                    gh = [
                        ghpool.tile([128, T], f32r, tag=f"g{mc}", name=f"g{mc}")
                        for mc in range(KT)
                    ] + [ghpool.tile([E, T], f32r, tag="h", name="h")]
                    pacc = [
                        psacc.tile([128, T], f32, tag="acc", name=f"acc{mc}")
                        for mc in range(KT)
                    ]
                    for k in range(NCHUNKS):
                        for mc in range(KT):
                            nc.tensor.matmul(
                                pacc[mc][:],
                                xr[k][:, ts(mc, 128)],
                                xr[k][:],
                                start=(k == 0),
                                stop=(k == NCHUNKS - 1),
                            )
                    for mc in range(KT):
                        nc.vector.tensor_copy(gh[mc][:], pacc[mc][:])
                    # H = e^T x as a dense second pass (e has landed by now)
                    ph = psacc.tile([128, T], f32, tag="acc", name="acch")
                    for k in range(NCHUNKS):
                        nc.tensor.matmul(
                            ph[:E, :],
                            er[:, k, :],
                            xr[k][:],
                            start=(k == 0),
                            stop=(k == NCHUNKS - 1),
                        )
                    nc.vector.tensor_copy(gh[KT][:], ph[:E, :])

                    # prefetch next batch's x while C/D run
                    if b + 1 < B2 or it + 1 < iters:
                        xr_b = load_x((b + 1) % B2)

                    # Stage C: M0 = W0m^T G + W1^T H ; M1 = W2^T G + W3^T H
                    m0 = [
                        mpool.tile([128, T], f32r, tag=f"m0{mc}", name=f"m0{mc}")
                        for mc in range(KT)
                    ]
                    m1 = mpool.tile([E, T], f32r, tag="m1")
                    for mc in range(KT):
                        pc = psacc.tile([128, T], f32, tag="acc")
                        # W0m is strictly upper triangular: block (kt, mc)
                        # is identically zero unless kt <= mc.
                        for kt in range(mc + 1):
                            nc.tensor.matmul(
                                pc[:],
                                w0mr[:, kt, ts(mc, 128)],
                                gh[kt][:],
                                start=(kt == 0),
                                stop=False,
                            )
                        nc.tensor.matmul(
                            pc[:], w1r[:, ts(mc, 128)], gh[KT][:],
                            start=False, stop=True,
                        )
                        nc.vector.tensor_copy(m0[mc][:], pc[:])
                    pc = psacc.tile([128, T], f32, tag="acc")
                    for kt in range(KT):
                        nc.tensor.matmul(
                            pc[:E, :], w2r[:, kt, :], gh[kt][:],
                            start=(kt == 0), stop=False,
                        )
                    nc.tensor.matmul(
                        pc[:E, :], w3r[:], gh[KT][:], start=False, stop=True
                    )
                    nc.vector.tensor_copy(m1[:], pc[:E, :])

                    # Stage D: out = x @ M0 + e @ M1, chunk by chunk over n.
                    for i in range(NCHUNKS):
                        ptx = pst.tile([128, 512], f32r, tag="pt")
                        for kt in range(KT):
                            nc.tensor.transpose(
                                ptx[:, ts(kt, 128)],
                                xr[i][:, ts(kt, 128)],
                                identr[:],
                            )
                        xt = xtpool.tile([128, KT, 128], f32r, tag="xt")
                        dst = xt[:].rearrange("p a q -> p (a q)")
                        if i % 2 == 0:
                            nc.vector.tensor_copy(dst, ptx[:])
                        else:
                            nc.scalar.activation(dst, ptx[:], AF.Copy)
                        po = psacc.tile([128, T], f32, tag="acc")
                        nc.tensor.matmul(
                            po[:],
                            etr[i // 4][:, ts(i % 4, 128)],
                            m1[:],
                            start=True,
                            stop=False,
                        )
                        for kt in range(KT):
                            nc.tensor.matmul(
                                po[:],
                                xt[:, kt, :],
                                m0[kt][:],
                                start=False,
                                stop=(kt == KT - 1),
                            )
                        ot = opool.tile([128, T], f32, tag="ot")
                        if i % 2 == 0:
                            nc.scalar.activation(ot[:], po[:], AF.Copy)
                        else:
                            nc.vector.tensor_copy(ot[:], po[:])
                        nc.sync.dma_start(OUT[b, ts(i, 128), :], ot[:])

    nc.compile()
    return nc


_CACHE = {}


def _get_module(iters: int = 1):
    if iters not in _CACHE:
        _CACHE[iters] = build_module(iters)
    return _CACHE[iters]


def _run(nc, in_maps, tries=3):
    last = None
    for _ in range(tries):
        try:
            return run_bass_kernel_spmd(nc, in_maps, list(range(N_CORES)))
        except Exception as ex:  # transient device wedges on first exec
            last = ex
    raise last


def kernel(x, e, W0, W1, W2, W3):
    nc = _get_module()
    x = np.ascontiguousarray(x, dtype=np.float32)
    in_maps = [
        {
            "x": x[c * B2 : (c + 1) * B2],
            "e": np.ascontiguousarray(e, dtype=np.float32),
            "W0": np.ascontiguousarray(W0, dtype=np.float32),
            "W1": np.ascontiguousarray(W1, dtype=np.float32),
            "W2": np.ascontiguousarray(W2, dtype=np.float32),
            "W3": np.ascontiguousarray(W3, dtype=np.float32),
        }
        for c in range(N_CORES)
    ]
    res = _run(nc, in_maps)
    out = np.concatenate([res.results[c]["out"] for c in range(N_CORES)], axis=0)
    return out



# revision 8
# speedup vs baseline: 1.0420x; 1.0420x over previous
"""Trainium2 Bass kernel for nn_Causal_Temporal_Map_Attention_2.

Reference computation (B=16, N=2048, T=512, E=64):
    W0m  = relu(triu(W0, 1))
    attn = (x@W0m.T)@x.T + (x@W1.T)@e.T + (e@W2.T)@x.T + (e@W3.T)@e.T
    out  = attn @ x

Associativity lets us avoid the [B, N, N] attention map entirely:
    G[b] = x[b].T @ x[b]                  # [512, 512]  (symmetric!)
    H[b] = e.T @ x[b]                     # [64, 512]
    M0[b] = W0m.T @ G[b] + W1.T @ H[b]    # [512, 512]
    M1[b] = W2.T @ G[b] + W3.T @ H[b]     # [64, 512]
    out[b] = x[b] @ M0[b] + e @ M1[b]     # [2048, 512]

Sharding: data-parallel over batch, 2 batches per core on 8 cores.

All matmuls run in bf16 (full PE rate at any free size; PSUM accumulates
in fp32; ~4e-3 max rel err, well under the 2e-2 gate).  G is computed
upper-triangular only (row-chunk mc covers cols >= 128*mc) and the lower
blocks are filled by PE transposes — 20480 matmul rows instead of 32768
per batch.  x^T for the final stage is built on the PE in a dedicated
phase between the Gram and M stages so the Tensor engine never idles.
"""

import numpy as np

import concourse.bass as bass
import concourse.mybir as mybir
import concourse.tile as tile
from concourse import bacc
from concourse.bass import ts
from concourse.bass_utils import run_bass_kernel_spmd
from concourse.masks import make_identity

N_CORES = 8
B = 16
B2 = B // N_CORES  # batches per core
N = 2048
T = 512
E = 64
NCHUNKS = N // 128  # 16
KT = T // 128  # 4

f32 = mybir.dt.float32
bf16 = mybir.dt.bfloat16
AF = mybir.ActivationFunctionType


def build_module():
    nc = bacc.Bacc("TRN2", target_bir_lowering=False, debug=False, num_devices=N_CORES)

    X = nc.dram_tensor("x", [B2, N, T], f32, kind="ExternalInput").ap()
    Ein = nc.dram_tensor("e", [N, E], f32, kind="ExternalInput").ap()
    W0 = nc.dram_tensor("W0", [T, T], f32, kind="ExternalInput").ap()
    W1 = nc.dram_tensor("W1", [E, T], f32, kind="ExternalInput").ap()
    W2 = nc.dram_tensor("W2", [T, E], f32, kind="ExternalInput").ap()
    W3 = nc.dram_tensor("W3", [E, E], f32, kind="ExternalInput").ap()
    OUT = nc.dram_tensor("out", [B2, N, T], f32, kind="ExternalOutput").ap()

    with tile.TileContext(nc) as tc:
        with (
            tc.tile_pool(name="const", bufs=1) as cpool,
            tc.tile_pool(name="wstage", bufs=1) as wpool,
            tc.tile_pool(name="xstage", bufs=16) as xspool,
            tc.tile_pool(name="xr", bufs=2) as xrpool,
            tc.tile_pool(name="xt", bufs=1) as xtpool,
            tc.tile_pool(name="gh", bufs=1) as ghpool,
            tc.tile_pool(name="m01", bufs=1) as mpool,
            tc.tile_pool(name="outst", bufs=2) as opool,
            tc.tile_pool(name="psacc", bufs=5, space="PSUM") as psacc,
            tc.tile_pool(name="pst", bufs=3, space="PSUM") as pst,
        ):
            # ---- x chunk DMAs alternate between the two HWDGE queues
            # (sync + scalar) so transfers ride both.  Issues are emitted
            # before casts so neither in-order engine queue stalls on a
            # transfer.  Casts go on vector/scalar; for prefetched batches
            # the caller emits them later (interleaved into stage D).
            def load_x_dma(b):
                xs = []
                for i in range(NCHUNKS):
                    x_st = xspool.tile([128, T], f32, tag="xs", name=f"xs{b}_{i}")
                    eng = nc.sync if i % 2 == 0 else nc.scalar
                    eng.dma_start(x_st[:], X[b, ts(i, 128), :])
                    xs.append(x_st)
                return xs

            def cast_x(xr, xs, i):
                if i % 2 == 0:
                    nc.vector.tensor_copy(xr[:, i, :], xs[i][:])
                else:
                    nc.scalar.activation(xr[:, i, :], xs[i][:], AF.Copy)

            # identity for PE transposes, first on the vector queue
            ident32 = cpool.tile([128, 128], f32)
            make_identity(nc, ident32[:])
            identb = cpool.tile([128, 128], bf16)
            nc.vector.tensor_copy(identb[:], ident32[:])

            # e first on the scalar queue: the eT transposes are the first
            # PE work and stage B's H matmul needs it at k=0.
            e_st = wpool.tile([128, NCHUNKS, E], f32)
            nc.scalar.dma_start(e_st[:], Ein.rearrange("(a p) k -> p a k", p=128))
            er = cpool.tile([128, NCHUNKS, E], bf16)
            nc.vector.tensor_copy(er[:], e_st[:])

            xs_b = load_x_dma(0)
            xr_b = xrpool.tile([128, NCHUNKS, T], bf16, tag="xr", name="xr0")
            for i in range(NCHUNKS):
                cast_x(xr_b, xs_b, i)

            # eT: [64, 2048] as 4 tiles of [64, 512] (lhsT for stage D's
            # e-term).  PE transposes; emitted before batch 0's stage B so
            # they warm the PE up while x is still streaming in.
            etr = [
                cpool.tile([E, 512], bf16, tag=f"etr{g}", name=f"etr{g}")
                for g in range(4)
            ]
            for g in range(4):
                pte = pst.tile([128, 512], bf16, tag="pt")
                for j in range(4):
                    i = g * 4 + j
                    nc.tensor.transpose(pte[:E, ts(j, 128)], er[:, i, :], identb[:])
                nc.vector.tensor_copy(etr[g][:], pte[:E, :])

            # ---- weights (needed from stage C on; DMAs queue up on sync
            # behind batch 0's even x chunks, casts behind the x casts)
            w0_st = wpool.tile([128, KT, T], f32)
            nc.sync.dma_start(w0_st[:], W0.rearrange("(a p) t -> p a t", p=128))
            w0m = cpool.tile([128, KT, T], bf16)
            for kt in range(KT):
                # keep W0[d, t] iff t >= d+1  (d = p + 128*kt)
                nc.gpsimd.affine_select(
                    out=w0_st[:, kt, :],
                    in_=w0_st[:, kt, :],
                    compare_op=mybir.AluOpType.is_ge,
                    fill=0.0,
                    base=-(128 * kt + 1),
                    pattern=[[1, T]],
                    channel_multiplier=-1,
                )
                nc.scalar.activation(w0m[:, kt, :], w0_st[:, kt, :], AF.Relu)

            w1_st = wpool.tile([E, T], f32)
            nc.sync.dma_start(w1_st[:], W1[:])
            w1b = cpool.tile([E, T], bf16)
            nc.vector.tensor_copy(w1b[:], w1_st[:])

            w2_st = wpool.tile([128, KT, E], f32)
            nc.sync.dma_start(w2_st[:], W2.rearrange("(a p) k -> p a k", p=128))
            w2b = cpool.tile([128, KT, E], bf16)
            nc.vector.tensor_copy(w2b[:], w2_st[:])

            w3_st = wpool.tile([E, E], f32)
            nc.sync.dma_start(w3_st[:], W3[:])
            w3b = cpool.tile([E, E], bf16)
            nc.vector.tensor_copy(w3b[:], w3_st[:])

            # ---------------- per batch ----------------
            for b in range(B2):
                xr = xr_b

                # Stage B: upper-triangular Gram.  Row-chunk mc covers
                # cols [128*mc, 512); 5 matmuls per arriving x chunk.
                pg = [
                    psacc.tile([128, T - 128 * mc], f32, tag="acc", name=f"pg{mc}")
                    for mc in range(KT)
                ]
                ph = psacc.tile([128, T], f32, tag="acc", name="ph")
                for k in range(NCHUNKS):
                    for mc in range(KT):
                        nc.tensor.matmul(
                            pg[mc][:],
                            xr[:, k, ts(mc, 128)],
                            xr[:, k, 128 * mc : T],
                            start=(k == 0),
                            stop=(k == NCHUNKS - 1),
                        )
                    nc.tensor.matmul(
                        ph[:E, :],
                        er[:, k, :],
                        xr[:, k, :],
                        start=(k == 0),
                        stop=(k == NCHUNKS - 1),
                    )

                # PSUM -> SBUF (cast to bf16), alternating engines
                gh = [
                    ghpool.tile([128, T], bf16, tag=f"g{mc}", name=f"g{mc}")
                    for mc in range(KT)
                ]
                h = ghpool.tile([E, T], bf16, tag="h", name="h")
                for mc in range(KT):
                    if mc % 2 == 0:
                        nc.vector.tensor_copy(gh[mc][:, 128 * mc : T], pg[mc][:])
                    else:
                        nc.scalar.activation(gh[mc][:, 128 * mc : T], pg[mc][:], AF.Copy)
                nc.vector.tensor_copy(h[:], ph[:E, :])

                # fill lower blocks of G by transposing the upper ones
                for a in range(KT):
                    for c in range(a + 1, KT):
                        ptg = pst.tile([128, 128], bf16, tag="pt")
                        nc.tensor.transpose(
                            ptg[:], gh[a][:, ts(c, 128)], identb[:]
                        )
                        if (a + c) % 2 == 0:
                            nc.vector.tensor_copy(gh[c][:, ts(a, 128)], ptg[:])
                        else:
                            nc.scalar.activation(gh[c][:, ts(a, 128)], ptg[:], AF.Copy)

                # XT phase: transpose all of x[b] for stage D.
                # xt[:, kt, i*128 + j] = x[b, i*128 + j, kt*128 + p]
                xt = xtpool.tile([128, KT, N], bf16, tag="xt", name="xt")
                for i in range(NCHUNKS):
                    ptx = pst.tile([128, 512], bf16, tag="pt")
                    for kt in range(KT):
                        nc.tensor.transpose(
                            ptx[:, ts(kt, 128)],
                            xr[:, i, ts(kt, 128)],
                            identb[:],
                        )
                    dst = xt[:, :, ts(i, 128)]
                    src = ptx[:].rearrange("p (a q) -> p a q", q=128)
                    if i % 2 == 0:
                        nc.vector.tensor_copy(dst, src)
                    else:
                        nc.scalar.activation(dst, src, AF.Copy)

                # prefetch next batch's x while C/D run (casts are
                # interleaved into stage D below, once transfers landed)
                if b + 1 < B2:
                    xs_b = load_x_dma(b + 1)
                    xr_b = xrpool.tile(
                        [128, NCHUNKS, T], bf16, tag="xr", name=f"xr{b + 1}"
                    )

                # Stage C: M0 = W0m^T G + W1^T H ; M1 = W2^T G + W3^T H
                m0 = mpool.tile([128, KT, T], bf16, tag="m0")
                m1 = mpool.tile([E, T], bf16, tag="m1")
                for mc in range(KT):
                    pc = psacc.tile([128, T], f32, tag="acc")
                    # W0m is strictly upper triangular: block (kt, mc)
                    # is identically zero unless kt <= mc.
                    for kt in range(mc + 1):
                        nc.tensor.matmul(
                            pc[:],
                            w0m[:, kt, ts(mc, 128)],
                            gh[kt][:],
                            start=(kt == 0),
                            stop=False,
                        )
                    nc.tensor.matmul(
                        pc[:], w1b[:, ts(mc, 128)], h[:], start=False, stop=True
                    )
                    if mc % 2 == 0:
                        nc.vector.tensor_copy(m0[:, mc, :], pc[:])
                    else:
                        nc.scalar.activation(m0[:, mc, :], pc[:], AF.Copy)
                pc = psacc.tile([128, T], f32, tag="acc")
                for kt in range(KT):
                    nc.tensor.matmul(
                        pc[:E, :], w2b[:, kt, :], gh[kt][:],
                        start=(kt == 0), stop=False,
                    )
                nc.tensor.matmul(pc[:E, :], w3b[:], h[:], start=False, stop=True)
                nc.vector.tensor_copy(m1[:], pc[:E, :])

                # Stage D: out = x @ M0 + e @ M1, chunk by chunk over n.
                # The e-term goes last in each accumulation group since m1
                # is the latest-ready operand.
                otg = None
                for i in range(NCHUNKS):
                    po = psacc.tile([128, T], f32, tag="acc")
                    for kt in range(KT):
                        nc.tensor.matmul(
                            po[:],
                            xt[:, kt, ts(i, 128)],
                            m0[:, kt, :],
                            start=(kt == 0),
                            stop=False,
                        )
                    nc.tensor.matmul(
                        po[:],
                        etr[i // 4][:, ts(i % 4, 128)],
                        m1[:],
                        start=False,
                        stop=True,
                    )
                    if i % 4 == 0:
                        otg = opool.tile([128, 4, T], f32, tag="ot")
                    if i % 2 == 0:
                        nc.scalar.activation(otg[:, i % 4, :], po[:], AF.Copy)
                    else:
                        nc.vector.tensor_copy(otg[:, i % 4, :], po[:])
                    # next batch's x casts ride along, one per chunk, on
                    # the engine opposite to the ot copy above
                    if b + 1 < B2:
                        cast_x(xr_b, xs_b, i)
                    if i % 4 == 3:
                        # one grouped store per 4 chunks, on the sync queue
                        nc.sync.dma_start(
                            OUT[b, 128 * (i - 3) : 128 * (i + 1), :].rearrange(
                                "(a p) t -> p a t", p=128
                            ),
                            otg[:],
                        )

    nc.compile()
    return nc


_CACHE = {}


def _get_module():
    if "m" not in _CACHE:
        _CACHE["m"] = build_module()
    return _CACHE["m"]


def _run(nc, in_maps, tries=3):
    last = None
    for _ in range(tries):
        try:
            return run_bass_kernel_spmd(nc, in_maps, list(range(N_CORES)))
        except Exception as ex:  # transient device wedges on first exec
            last = ex
    raise last


def kernel(x, e, W0, W1, W2, W3):
    nc = _get_module()
    x = np.ascontiguousarray(x, dtype=np.float32)
    in_maps = [
        {
            "x": x[c * B2 : (c + 1) * B2],
            "e": np.ascontiguousarray(e, dtype=np.float32),
            "W0": np.ascontiguousarray(W0, dtype=np.float32),
            "W1": np.ascontiguousarray(W1, dtype=np.float32),
            "W2": np.ascontiguousarray(W2, dtype=np.float32),
            "W3": np.ascontiguousarray(W3, dtype=np.float32),
        }
        for c in range(N_CORES)
    ]
    res = _run(nc, in_maps)
    out = np.concatenate([res.results[c]["out"] for c in range(N_CORES)], axis=0)
    return out
